# revision 9
# baseline (speedup 1.0000x reference)
"""Self-contained Trainium2 Bass kernel for the DiT forward pass.

Data-parallel over batch: 16 images -> 8 cores x 2 images. Each core runs the
full transformer on its 2048 tokens, processing one image at a time through
each block phase. GEMMs run in fp32r (full-rate fp32 on the PE) except
attention and fc2 which use bf16 operands. The residual stream lives in DRAM
and is streamed per 128-token tile.
"""
import math
import os
import sys

for _p in ("/opt/trn_rl_repo", os.path.expanduser("~/.axon_site/_ro/trn_rl_repo")):
    if os.path.isdir(_p) and _p not in sys.path:
        sys.path.insert(0, _p)

import numpy as np
import ml_dtypes

import concourse.bass as bass
import concourse.mybir as mybir
import concourse.tile as tile
from concourse import bacc
from concourse.bass import ts, ds
from concourse.bass_utils import run_bass_kernel_spmd

F32 = mybir.dt.float32
F32R = mybir.dt.float32r
BF16 = mybir.dt.bfloat16
AF = mybir.ActivationFunctionType
ALU = mybir.AluOpType

# model dims
B, IMG, PS, HID, HEADS, DEPTH, MLP, FREQ, NCLS, OC = 16, 256, 8, 768, 12, 12, 3072, 256, 1000, 3
G = IMG // PS           # 32
NTOK = G * G            # 1024 tokens per image
NCORES = 8
BPC = B // NCORES       # 2 images/core
T = BPC * NTOK          # 2048 tokens/core
HD = HID // HEADS       # 64
PIN = 3 * PS * PS       # 192
KH = HID // 128         # 6
MT = T // 128           # 16
MI = NTOK // 128        # 8 token tiles per image
KM = MLP // 128         # 24

NSPLIT = ((0, 512), (512, 256))  # 768 = 512 + 256


def build_program(depth=DEPTH):
    nc = bacc.Bacc()
    D = {}

    def din(name, shape, dt=F32):
        D[name] = nc.declare_dram_parameter(name, list(shape), dt, isOutput=False)
        return D[name]

    din("xpT", (PIN, T))
    din("tv", (1, BPC))
    din("dtv", (1, BPC))
    din("ye", (BPC, HID))
    din("pos", (NTOK, HID))
    din("freqs", (1, FREQ // 2))
    din("identc", (128, 128))
    din("onesf", (1, 128))
    din("onesbf", (1, 128), BF16)
    din("onescol", (128, 1), BF16)
    din("epshp", (128, 2))
    din("pwT", (PIN, HID)); din("pb", (1, HID))
    for p in ("te", "dte"):
        din(p + "W1T", (FREQ, HID)); din(p + "B1", (1, HID))
        din(p + "W2T", (HID, HID)); din(p + "B2", (1, HID))
    din("cwT", (depth, HID, 6 * HID)); din("cb", (depth, 1, 6 * HID))
    din("qwT", (depth, HID, HID)); din("qb", (depth, 128, KH))
    din("kwT", (depth, HID, HID)); din("kb", (depth, 128, KH))
    din("vwT", (depth, HID, HID)); din("vb", (depth, 1, HID))
    din("awT", (depth, HID, HID), BF16); din("ab", (depth, 1, HID), BF16)
    din("w1T", (depth, HID, MLP)); din("b1", (depth, 128, KM))
    din("w2T", (depth, MLP, HID), BF16); din("b2", (depth, 1, HID), BF16)
    din("fcwT", (HID, 2 * HID)); din("fcb", (1, 2 * HID))
    din("fwT", (HID, PIN)); din("fb", (PIN, 1))
    outT = nc.declare_dram_parameter("outT", [PIN, T], F32, isOutput=True)

    from contextlib import ExitStack
    with tile.TileContext(nc) as tc, ExitStack() as es:
        cst = es.enter_context(tc.tile_pool(name="cst", bufs=1))
        per = es.enter_context(tc.tile_pool(name="per", bufs=1))
        big = es.enter_context(tc.tile_pool(name="big", bufs=1))
        vap = es.enter_context(tc.tile_pool(name="vap", bufs=1))
        wq2 = es.enter_context(tc.tile_pool(name="wq2", bufs=2))
        wq4 = es.enter_context(tc.tile_pool(name="wq4", bufs=4))
        sp2 = es.enter_context(tc.tile_pool(name="sp2", bufs=2))
        xhp = es.enter_context(tc.tile_pool(name="xhp", bufs=1))
        xtp = es.enter_context(tc.tile_pool(name="xtp", bufs=3))
        xop = es.enter_context(tc.tile_pool(name="xop", bufs=2))
        tny = es.enter_context(tc.tile_pool(name="tny", bufs=2))
        cnd = es.enter_context(tc.tile_pool(name="cnd", bufs=2))
        ccp = es.enter_context(tc.tile_pool(name="ccp", bufs=2))
        tn1 = es.enter_context(tc.tile_pool(name="tn1", bufs=1))
        sp1 = es.enter_context(tc.tile_pool(name="sp1", bufs=1))
        dnp = es.enter_context(tc.tile_pool(name="dnp", bufs=1))
        vwp = es.enter_context(tc.tile_pool(name="vwp", bufs=1))
        xdp = es.enter_context(tc.tile_pool(name="xdp", bufs=1, space="DRAM"))
        psp = es.enter_context(tc.tile_pool(name="psp", bufs=4, space="PSUM"))

        # ---------- constants ----------
        ident = cst.tile([128, 128], F32)
        nc.sync.dma_start(out=ident[:], in_=D["identc"][:])
        onesf = cst.tile([1, 128], F32R)
        nc.sync.dma_start(out=onesf[:], in_=D["onesf"][:].bitcast(F32R))
        onesbf = cst.tile([1, 128], BF16)
        nc.sync.dma_start(out=onesbf[:], in_=D["onesbf"][:])
        onescol = cst.tile([128, 1], BF16)
        nc.sync.dma_start(out=onescol[:], in_=D["onescol"][:])
        epshp = cst.tile([128, 2], F32)
        nc.sync.dma_start(out=epshp[:], in_=D["epshp"][:])
        eps_c = epshp[:, 0:1]
        hpi_c = epshp[:, 1:2]
        fr = cst.tile([1, 128], F32R)
        nc.sync.dma_start(out=fr[:], in_=D["freqs"][:].bitcast(F32R))

        # ---------- persistent ----------
        xmT = per.tile([128, KH, NTOK], F32R)       # per-image Xm^T / Xm2^T
        aT = per.tile([128, KH, NTOK], BF16)        # per-image attn out (transposed)
        csT = per.tile([128, KH, BPC], F32R)
        cc2T = per.tile([128, 12, BPC], F32)
        stats = per.tile([128, MI, 2, 6], F32)
        mv = per.tile([128, MI, 2], F32)
        rstd = per.tile([128, MI], F32)
        nmr = per.tile([128, MI], F32)
        xxd = xdp.tile([128, MT, HID], F32)         # residual stream (DRAM)

        def mm(out, lhsT, rhs, start, stop):
            nc.tensor.matmul(out, lhsT, rhs, start=start, stop=stop)

        def wtile(dram_ap, shape, dt=F32R, tag="wf32"):
            t = wq4.tile(list(shape), dt, tag=tag, name="wt_" + tag)
            src = dram_ap.bitcast(dt) if dt == F32R else dram_ap
            nc.sync.dma_start(out=t[:], in_=src)
            return t

        # ---------- patch embed ----------
        pbr = tny.tile([1, HID], F32R, tag="rowr", name="pbr")
        nc.sync.dma_start(out=pbr[:], in_=D["pb"][:].bitcast(F32R))
        for mt in range(MT):
            prow = mt % MI
            xqa = wtile(D["xpT"][0:128, ts(mt, 128)], (128, 128), tag="wf32")
            xqb = wtile(D["xpT"][128:192, ts(mt, 128)], (64, 128), tag="wf32")
            pst = sp1.tile([128, 768], F32, tag="pst")
            nc.sync.dma_start(out=pst[:], in_=D["pos"][ts(prow, 128), :])
            for n0, nw in NSPLIT:
                pwt = wtile(D["pwT"][0:128, n0:n0 + nw], (128, nw), tag="wf32")
                pwu = wtile(D["pwT"][128:192, n0:n0 + nw], (64, nw), tag="wf32")
                ps = psp.tile([128, nw], F32, tag="ps")
                mm(ps[:], onesf[:, 0:128], pbr[:, n0:n0 + nw], True, False)
                mm(ps[:], xqa[:], pwt[:], False, False)
                mm(ps[:], xqb[:], pwu[:], False, True)
                xo = xop.tile([128, 512], F32, tag="xo2", name="xo")
                nc.vector.tensor_tensor(out=xo[:, 0:nw], in0=ps[:],
                                        in1=pst[:, n0:n0 + nw], op=ALU.add)
                nc.sync.dma_start(out=xxd[:, mt, n0:n0 + nw], in_=xo[:, 0:nw])

        # ---------- conditioning ----------
        def transpose_2w(src_ap, dst_f32r, width):
            for k in range(width // 128):
                tp = psp.tile([128, BPC], F32, tag="ps", name="tp")
                nc.tensor.transpose(tp[:], src_ap[:, ts(k, 128)], ident[0:BPC, 0:BPC])
                nc.scalar.copy(dst_f32r[:, k, :], tp[:])

        def temb_mlp(tname, w1, b1n, w2, b2n, cps, first):
            tvs = tn1.tile([1, BPC], F32R, tag="tvs")
            nc.sync.dma_start(out=tvs[:], in_=D[tname][:].bitcast(F32R))
            args = psp.tile([BPC, 128], F32, tag="ps")
            mm(args[:], tvs[:], fr[:], True, True)
            emb = tn1.tile([BPC, FREQ], F32, tag="emb")
            nc.scalar.activation(emb[:, 0:128], args[:], AF.Sin, bias=hpi_c[0:BPC, :],
                                 scale=1.0)
            nc.scalar.activation(emb[:, 128:256], args[:], AF.Sin, bias=0.0, scale=1.0)
            embT = cnd.tile([128, 2, BPC], F32R, tag="embT")
            transpose_2w(emb[:], embT, FREQ)
            b1r = tny.tile([1, HID], F32R, tag="rowr", name="b1r")
            nc.sync.dma_start(out=b1r[:], in_=D[b1n][:].bitcast(F32R))
            h1 = psp.tile([BPC, HID], F32, tag="ps")
            for n0, nw in NSPLIT:
                mm(h1[:, n0:n0 + nw], onesf[:, 0:BPC], b1r[:, n0:n0 + nw], True, False)
                for k in range(2):
                    w1s = wtile(D[w1][ts(k, 128), n0:n0 + nw], (128, nw), tag="wf32")
                    mm(h1[:, n0:n0 + nw], embT[:, k, :], w1s[:], False, k == 1)
            h1s = cnd.tile([BPC, HID], F32, tag="crow", name="h1s")
            nc.scalar.activation(h1s[:], h1[:], AF.Silu)
            h1sT = cnd.tile([128, KH, BPC], F32R, tag="h1sT")
            transpose_2w(h1s[:], h1sT, HID)
            b2r = tny.tile([1, HID], F32R, tag="rowr", name="b2r")
            nc.sync.dma_start(out=b2r[:], in_=D[b2n][:].bitcast(F32R))
            for n0, nw in NSPLIT:
                mm(cps[:, n0:n0 + nw], onesf[:, 0:BPC], b2r[:, n0:n0 + nw], first, False)
                for k in range(KH):
                    w2s = wtile(D[w2][ts(k, 128), n0:n0 + nw], (128, nw), tag="wf32")
                    mm(cps[:, n0:n0 + nw], h1sT[:, k, :], w2s[:],
                       False, (not first) and k == KH - 1)

        cps = psp.tile([BPC, HID], F32, tag="ps", name="cps")
        temb_mlp("tv", "teW1T", "teB1", "teW2T", "teB2", cps, True)
        temb_mlp("dtv", "dteW1T", "dteB1", "dteW2T", "dteB2", cps, False)
        yesb = cnd.tile([BPC, HID], F32, tag="crow", name="yesb")
        nc.sync.dma_start(out=yesb[:], in_=D["ye"][:])
        csb = cnd.tile([BPC, HID], F32, tag="crow", name="csb")
        nc.vector.tensor_tensor(out=csb[:], in0=cps[:], in1=yesb[:], op=ALU.add)
        cssb = cnd.tile([BPC, HID], F32, tag="crow", name="cssb")
        nc.scalar.activation(cssb[:], csb[:], AF.Silu)
        transpose_2w(cssb[:], csT, HID)

        def small_gemm_T(wname, bname, nblocks, outT_tile, plus1_chunks):
            for n in range(nblocks):
                cbr = tn1.tile([1, 512], F32R, tag="cbr")
                nc.sync.dma_start(out=cbr[:], in_=bname[:, ts(n, 512)].bitcast(F32R))
                ps = psp.tile([BPC, 512], F32, tag="ps")
                mm(ps[:], onesf[:, 0:BPC], cbr[:], True, False)
                for k in range(KH):
                    wt = wtile(wname[ts(k, 128), ts(n, 512)], (128, 512), tag="wf32")
                    mm(ps[:], csT[:, k, :], wt[:], False, k == KH - 1)
                sb = cnd.tile([BPC, 512], F32, tag="ccstage", name="sb")
                nc.scalar.copy(sb[:], ps[:])
                for j in range(4):
                    tp = psp.tile([128, BPC], F32, tag="ps", name="tp")
                    nc.tensor.transpose(tp[:], sb[:, ts(j, 128)], ident[0:BPC, 0:BPC])
                    nc.scalar.copy(outT_tile[:, n * 4 + j, :], tp[:])
            for ch0 in plus1_chunks:
                nc.vector.tensor_scalar_add(outT_tile[:, ch0:ch0 + KH, :],
                                            outT_tile[:, ch0:ch0 + KH, :], 1.0)

        small_gemm_T(D["fcwT"], D["fcb"], 3, cc2T, (KH,))
        ccT_cur = ccp.tile([128, 36, BPC], F32, tag="ccT", name="ccT0")
        small_gemm_T(D["cwT"][0], D["cb"][0], 9, ccT_cur, (KH, 4 * KH))

        # ---------- LN + modulate + transpose (one image) ----------
        def ln_modulate(img, modT, sh_base, sc_base):
            for i in range(MI):
                xt = xtp.tile([128, HID], F32, tag="xt", name="xt_s")
                nc.sync.dma_start(out=xt[:], in_=xxd[:, img * MI + i, :])
                nc.vector.bn_stats(stats[:, i, 0, :], xt[:, 0:384])
                nc.vector.bn_stats(stats[:, i, 1, :], xt[:, 384:768])
            for i in range(MI):
                nc.vector.bn_aggr(mv[:, i, :], stats[:, i, :, :])
            nc.scalar.activation(rstd[:], mv[:, :, 1], AF.Sqrt, bias=eps_c, scale=1.0)
            nc.vector.reciprocal(rstd[:], rstd[:])
            nc.vector.scalar_tensor_tensor(out=nmr[:], in0=mv[:, :, 0], scalar=-1.0,
                                           in1=rstd[:], op0=ALU.mult, op1=ALU.mult)
            for mg in range(MI // 2):
                xh = xhp.tile([128, 2, HID], F32, tag="xh")
                for j in range(2):
                    m = mg * 2 + j
                    xt = xtp.tile([128, HID], F32, tag="xt", name="xt_h")
                    nc.sync.dma_start(out=xt[:], in_=xxd[:, img * MI + m, :])
                    nc.scalar.activation(xh[:, j, :], xt[:], AF.Identity,
                                         bias=nmr[:, m:m + 1], scale=rstd[:, m:m + 1])
                for f in range(KH):
                    tp = psp.tile([128, 256], F32, tag="ps", name="tp2")
                    for j in range(2):
                        nc.tensor.transpose(tp[:, ts(j, 128)], xh[:, j, ts(f, 128)],
                                            ident[:])
                    nc.scalar.activation(xmT[:, f, mg * 256:(mg + 1) * 256], tp[:],
                                         AF.Identity,
                                         bias=modT[:, sh_base + f, img:img + 1],
                                         scale=modT[:, sc_base + f, img:img + 1])

        def gate_bcast(ccT_tile, base, img):
            grow = tn1.tile([1, HID], F32, tag="grow")
            for ch in range(KH):
                nc.sync.dma_start(out=grow[0:1, ts(ch, 128)],
                                  in_=ccT_tile[:, base + ch, img:img + 1])
            gb = sp1.tile([128, HID], F32, tag="pst", name="gb")
            nc.gpsimd.partition_broadcast(gb[:], grow[:])
            return gb

        def gated_residual(img, mt, gb, ps, n0, nw):
            xt = xtp.tile([128, HID], F32, tag="xt", name="xt_r")
            nc.sync.dma_start(out=xt[:, 0:nw], in_=xxd[:, img * MI + mt, n0:n0 + nw])
            tmp = sp2.tile([128, 512], F32, tag="tmp")
            nc.vector.tensor_tensor(out=tmp[:, 0:nw], in0=ps, in1=gb[:, n0:n0 + nw],
                                    op=ALU.mult)
            xo = xop.tile([128, 512], F32, tag="xo2")
            nc.vector.tensor_tensor(out=xo[:, 0:nw], in0=xt[:, 0:nw], in1=tmp[:, 0:nw],
                                    op=ALU.add)
            nc.sync.dma_start(out=xxd[:, img * MI + mt, n0:n0 + nw], in_=xo[:, 0:nw])

        # ---------- blocks ----------
        for d in range(depth):
            qbs = tn1.tile([128, KH], F32, tag="qbs")
            nc.sync.dma_start(out=qbs[:], in_=D["qb"][d])
            kbs = tn1.tile([128, KH], F32, tag="kbs")
            nc.sync.dma_start(out=kbs[:], in_=D["kb"][d])
            vbr = tny.tile([1, HID], F32R, tag="rowr", name="vbr")
            nc.sync.dma_start(out=vbr[:], in_=D["vb"][d].bitcast(F32R))
            awt = big.tile([128, KH, HID], BF16, tag="bigA", name="awt")
            for k in range(KH):
                nc.sync.dma_start(out=awt[:, k, :], in_=D["awT"][d, ts(k, 128), :])
            abr = tn1.tile([1, HID], BF16, tag="rowb", name="abr")
            nc.sync.dma_start(out=abr[:], in_=D["ab"][d])
            vwt = vwp.tile([128, KH, HID], F32R, tag="vwt")
            for k in range(KH):
                nc.sync.dma_start(out=vwt[:, k, :],
                                  in_=D["vwT"][d, ts(k, 128), :].bitcast(F32R))

            for img in range(BPC):
                ln_modulate(img, ccT_cur, 0, KH)

                # V GEMM -> vA  [128, kt, head, 65]
                vA = vap.tile([128, MI, HEADS, HD + 1], BF16, tag="vA")
                for mt in range(MI):
                    for n0, nw in NSPLIT:
                        ps = psp.tile([128, nw], F32, tag="ps")
                        mm(ps[:], onesf[:, 0:128], vbr[:, n0:n0 + nw], True, False)
                        for k in range(KH):
                            mm(ps[:], xmT[:, k, ts(mt, 128)], vwt[:, k, n0:n0 + nw],
                               False, k == KH - 1)
                        h0 = n0 // HD
                        nc.vector.tensor_copy(out=vA[:, mt, h0:h0 + nw // HD, 0:HD],
                                              in_=ps[:])
                nc.vector.memset(vA[:, :, :, HD:HD + 1], 1.0)

                for f in range(KH):
                    qwf = wq2.tile([128, KH, 128], F32R, tag="qkwf", name="qwf")
                    nc.sync.dma_start(
                        out=qwf[:],
                        in_=D["qwT"][d, :, ts(f, 128)].bitcast(F32R).rearrange(
                            "(a p) n -> p a n", p=128))
                    kwf = wq2.tile([128, KH, 128], F32R, tag="qkwf", name="kwf")
                    nc.sync.dma_start(
                        out=kwf[:],
                        in_=D["kwT"][d, :, ts(f, 128)].bitcast(F32R).rearrange(
                            "(a p) n -> p a n", p=128))
                    qTf = sp1.tile([128, NTOK], BF16, tag="qTf")
                    kTf = sp1.tile([128, NTOK], BF16, tag="kTf")
                    for dst, wf, bb in ((qTf, qwf, qbs), (kTf, kwf, kbs)):
                        for n in range(2):
                            ps = psp.tile([128, 512], F32, tag="ps")
                            for k in range(KH):
                                mm(ps[:], wf[:, k, :], xmT[:, k, ts(n, 512)],
                                   k == 0, k == KH - 1)
                            nc.scalar.activation(dst[:, ts(n, 512)], ps[:], AF.Identity,
                                                 bias=bb[:, f:f + 1], scale=1.0)
                    for h in (2 * f, 2 * f + 1):
                        odd = h % 2
                        po = 64 * odd
                        oT = psp.tile([128, NTOK], F32, tag="ps", name="oT")
                        den = None
                        if odd:
                            den = psp.tile([1, NTOK], F32, tag="ps", name="den")

                        def av(se, kt, last):
                            for qs in range(2):
                                if odd:
                                    mm(oT[64:128, ts(qs, 512)], vA[:, kt, h, 0:HD],
                                       se[:, ts(qs, 512)], kt == 0, last)
                                    mm(den[:, ts(qs, 512)], onescol[:],
                                       se[:, ts(qs, 512)], kt == 0, last)
                                else:
                                    mm(oT[0:65, ts(qs, 512)], vA[:, kt, h, 0:HD + 1],
                                       se[:, ts(qs, 512)], kt == 0, last)

                        prev = None
                        for kt in range(MI):
                            sp = psp.tile([128, NTOK], F32, tag="ps", name="sp")
                            for qs in range(2):
                                mm(sp[:, ts(qs, 512)], kTf[po:po + 64, ts(kt, 128)],
                                   qTf[po:po + 64, ts(qs, 512)], True, True)
                            se = sp2.tile([128, NTOK], BF16, tag="se", name="se")
                            nc.scalar.activation(se[:], sp[:], AF.Exp)
                            if prev is not None:
                                av(prev[0], prev[1], False)
                            prev = (se, kt)
                        av(prev[0], prev[1], True)
                        dn = dnp.tile([128, NTOK], F32, tag="dn", name="dn")
                        dr = dn[po:po + 1, :]
                        nc.scalar.copy(dr, den[0:1, :] if odd else oT[64:65, :])
                        nc.vector.reciprocal(dr, dr)
                        rdb = dnp.tile([128, NTOK], F32, tag="rdb")
                        nc.gpsimd.partition_broadcast(rdb[:], dr)
                        nc.vector.tensor_tensor(
                            out=aT[po:po + 64, f, :],
                            in0=oT[po:po + 64, :], in1=rdb[po:po + 64, :], op=ALU.mult)

                # attn out proj + gated residual
                gb = gate_bcast(ccT_cur, 2 * KH, img)
                for mt in range(MI):
                    for n0, nw in NSPLIT:
                        ps = psp.tile([128, nw], F32, tag="ps")
                        mm(ps[:], onesbf[:], abr[:, n0:n0 + nw], True, False)
                        for k in range(KH):
                            mm(ps[:], aT[:, k, ts(mt, 128)], awt[:, k, n0:n0 + nw],
                               False, k == KH - 1)
                        gated_residual(img, mt, gb, ps[:], n0, nw)

            # next block's modulation vectors (overlaps with MLP)
            ccT_next = None
            if d + 1 < depth:
                ccT_next = ccp.tile([128, 36, BPC], F32, tag="ccT", name="ccTn")
                small_gemm_T(D["cwT"][d + 1], D["cb"][d + 1], 9, ccT_next,
                             (KH, 4 * KH))

            # MLP per image
            b1s = tn1.tile([128, KM], F32, tag="b1s")
            nc.sync.dma_start(out=b1s[:], in_=D["b1"][d])
            b2r = tn1.tile([1, HID], BF16, tag="rowb", name="b2r")
            nc.sync.dma_start(out=b2r[:], in_=D["b2"][d])
            for img in range(BPC):
                ln_modulate(img, ccT_cur, 3 * KH, 4 * KH)
                gb = gate_bcast(ccT_cur, 5 * KH, img)
                fc1T = big.tile([128, KM, NTOK], BF16, tag="bigA", name="fc1T")
                for mo in range(KM):
                    w1t = wq2.tile([128, KH, 128], F32R, tag="qkwf", name="w1t")
                    nc.sync.dma_start(
                        out=w1t[:],
                        in_=D["w1T"][d, :, ts(mo, 128)].bitcast(F32R).rearrange(
                            "(a p) n -> p a n", p=128))
                    for n in range(2):
                        ps = psp.tile([128, 512], F32, tag="ps")
                        for k in range(KH):
                            mm(ps[:], w1t[:, k, :], xmT[:, k, ts(n, 512)],
                               k == 0, k == KH - 1)
                        nc.scalar.activation(fc1T[:, mo, ts(n, 512)], ps[:], AF.Gelu,
                                             bias=b1s[:, mo:mo + 1], scale=1.0)
                for sub in range(2):
                    for n0, nw in NSPLIT:
                        pss = []
                        for mt in range(4):
                            ps = psp.tile([128, nw], F32, tag="ps", name="ps_mt")
                            mm(ps[:], onesbf[:], b2r[:, n0:n0 + nw], True, False)
                            pss.append(ps)
                        for k2 in range(KM):
                            w2t = wq4.tile([128, 512], BF16, tag="w2t")
                            nc.sync.dma_start(out=w2t[:, 0:nw],
                                              in_=D["w2T"][d, ts(k2, 128), n0:n0 + nw])
                            for mt in range(4):
                                mm(pss[mt][:],
                                   fc1T[:, k2, ds(sub * 512 + mt * 128, 128)],
                                   w2t[:, 0:nw], False, k2 == KM - 1)
                        for mt in range(4):
                            gated_residual(img, sub * 4 + mt, gb, pss[mt][:], n0, nw)
            if ccT_next is not None:
                ccT_cur = ccT_next

        # ---------- final layer ----------
        fbs = tn1.tile([128, 2], F32, tag="fbs")
        nc.sync.dma_start(out=fbs[:, 0:1], in_=D["fb"][0:128, :])
        nc.sync.dma_start(out=fbs[0:64, 1:2], in_=D["fb"][128:192, :])
        for img in range(BPC):
            ln_modulate(img, cc2T, 0, KH)
            for n in range(2):
                ps = psp.tile([128, 512], F32, tag="ps")
                for k in range(KH):
                    fwa = wtile(D["fwT"][ts(k, 128), 0:128], (128, 128), tag="wf32")
                    mm(ps[:], fwa[:], xmT[:, k, ts(n, 512)], k == 0, k == KH - 1)
                ob = sp2.tile([128, 512], F32, tag="tmp", name="ob")
                nc.scalar.activation(ob[:], ps[:], AF.Identity, bias=fbs[:, 0:1],
                                     scale=1.0)
                nc.sync.dma_start(out=outT[0:128, ds(img * NTOK + n * 512, 512)],
                                  in_=ob[:])
                ps2 = psp.tile([64, 512], F32, tag="ps", name="ps2")
                for k in range(KH):
                    fwb = wtile(D["fwT"][ts(k, 128), 128:192], (128, 64), tag="wf32")
                    mm(ps2[:], fwb[:], xmT[:, k, ts(n, 512)], k == 0, k == KH - 1)
                ob2 = sp2.tile([64, 512], F32, tag="tmp", name="ob2")
                nc.scalar.activation(ob2[:], ps2[:], AF.Identity, bias=fbs[0:64, 1:2],
                                     scale=1.0)
                nc.sync.dma_start(out=outT[128:192, ds(img * NTOK + n * 512, 512)],
                                  in_=ob2[:])

    nc.compile()
    return nc


# ---------------------------------------------------------------- host side
def _pos_embed_2d(dim, num_patches):
    g = int(math.isqrt(num_patches))
    def emb1d(d, pos):
        omega = np.arange(d // 2, dtype=np.float32) / (d / 2.0)
        omega = 1.0 / (10000.0 ** omega)
        out = pos.reshape(-1)[:, None] * omega[None]
        return np.concatenate([np.sin(out), np.cos(out)], -1)
    gw, gh = np.meshgrid(np.arange(g, dtype=np.float32), np.arange(g, dtype=np.float32))
    return np.concatenate([emb1d(dim // 2, gw), emb1d(dim // 2, gh)], -1).astype(np.float32)


def host_inputs(x, t, dt, y, params, depth=DEPTH):
    p = {k: np.asarray(v) for k, v in params.items() if k != "blocks"}
    bl = {k: np.asarray(v) for k, v in params["blocks"].items()}
    f32 = lambda a: np.ascontiguousarray(a, dtype=np.float32)
    bf = lambda a: np.ascontiguousarray(np.asarray(a, np.float32).astype(ml_dtypes.bfloat16))

    shared = {
        "pos": f32(_pos_embed_2d(HID, NTOK)),
        "freqs": f32(np.exp(-math.log(10000.0) * np.arange(FREQ // 2) / (FREQ // 2))[None]),
        "identc": np.eye(128, dtype=np.float32),
        "onesf": np.ones((1, 128), np.float32),
        "onesbf": np.ones((1, 128), ml_dtypes.bfloat16),
        "onescol": np.ones((128, 1), ml_dtypes.bfloat16),
        "epshp": np.tile(np.array([[1e-5, math.pi / 2]], np.float32), (128, 1)),
        "pwT": f32(p["patch_w"].reshape(HID, PIN).T),
        "pb": f32(p["patch_b"][None]),
        "teW1T": f32(p["te_w1"].T), "teB1": f32(p["te_b1"][None]),
        "teW2T": f32(p["te_w2"].T), "teB2": f32(p["te_b2"][None]),
        "dteW1T": f32(p["dte_w1"].T), "dteB1": f32(p["dte_b1"][None]),
        "dteW2T": f32(p["dte_w2"].T), "dteB2": f32(p["dte_b2"][None]),
        "cwT": f32(bl["c_w"][:depth].transpose(0, 2, 1)),
        "cb": f32(bl["c_b"][:depth, None, :]),
        "qwT": f32(bl["q_w"][:depth].transpose(0, 2, 1) / HD),
        "qb": f32((bl["q_b"][:depth] / HD).reshape(depth, KH, 128).transpose(0, 2, 1)),
        "kwT": f32(bl["k_w"][:depth].transpose(0, 2, 1)),
        "kb": f32(bl["k_b"][:depth].reshape(depth, KH, 128).transpose(0, 2, 1)),
        "vwT": f32(bl["v_w"][:depth].transpose(0, 2, 1)),
        "vb": f32(bl["v_b"][:depth, None, :]),
        "awT": bf(bl["a_w"][:depth].transpose(0, 2, 1)),
        "ab": bf(bl["a_b"][:depth, None, :]),
        "w1T": f32(bl["m_w1"][:depth].transpose(0, 2, 1)),
        "b1": f32(bl["m_b1"][:depth].reshape(depth, KM, 128).transpose(0, 2, 1)),
        "w2T": bf(bl["m_w2"][:depth].transpose(0, 2, 1)),
        "b2": bf(bl["m_b2"][:depth, None, :]),
        "fcwT": f32(p["f_c_w"].T), "fcb": f32(p["f_c_b"][None]),
        "fwT": f32(p["f_w"].T), "fb": f32(p["f_b"][:, None]),
    }
    x = np.asarray(x, np.float32)
    xp = x.reshape(B, 3, G, PS, G, PS).transpose(0, 2, 4, 1, 3, 5).reshape(B, NTOK, PIN)
    ye_all = np.asarray(params["label_emb"])[np.asarray(y)]
    in_maps = []
    for i in range(NCORES):
        s = slice(i * BPC, (i + 1) * BPC)
        m = dict(shared)
        m["xpT"] = f32(xp[s].reshape(T, PIN).T)
        m["tv"] = f32(np.asarray(t)[None, s])
        m["dtv"] = f32(np.asarray(dt)[None, s])
        m["ye"] = f32(ye_all[s])
        in_maps.append(m)
    return in_maps


def assemble_output(results):
    out = np.empty((B, OC, IMG, IMG), np.float32)
    for i in range(NCORES):
        oT = results[i]["outT"]  # [192, 2048]
        for b in range(BPC):
            o = oT[:, b * NTOK:(b + 1) * NTOK].T  # [1024, 192]
            o = o.reshape(G, G, PS, PS, OC).transpose(4, 0, 2, 1, 3).reshape(OC, IMG, IMG)
            out[i * BPC + b] = o
    return out


_prog = None


def kernel(x, t, dt, y, params):
    global _prog
    if _prog is None:
        _prog = build_program(DEPTH)
    in_maps = host_inputs(x, t, dt, y, params, DEPTH)
    res = run_bass_kernel_spmd(_prog, in_maps, core_ids=list(range(NCORES)))
    return assemble_output(res.results)
